# revision 39
# baseline (speedup 1.0000x reference)
"""NT-Xent contrastive loss (SimCLR) on 8 Trainium2 NeuronCores.

Strategy (v6: host-norm + fp8 DoubleRow + 128-granular symmetry):
  - Host: z = concat(z_i, z_j) [8192, 1024], L2-normalize rows in f32,
    scale by S=16, quantize to fp8 e4m3. The cosine-sim matrix is then
    just G = q @ q.T (scaled by S^2), no on-device normalization.
  - Symmetry: sim is symmetric. In rotated coords (each core's 1024 rows
    at block 0), core c computes column blocks 1-3 in full; block 0 (its
    own diagonal block) as a TRUE upper triangle at 128-col granularity
    (m-tile m computes cols >= 128*m, in ragged 512/384/256/128-col
    chunks whose boundaries stay on the PSUM bank line at col 512 -- a
    matmul output straddling a bank boundary is silently corrupted); and
    block 4 as HALF a block: the (c, c+4) / (c+4, c) pair is split at
    512-supertile granularity - cores 0-3 compute the two diagonal
    supertiles, cores 4-7 the two anti-diagonal ones. SPMD runs ONE
    program on all cores, so that split is realized in DATA: the host
    swaps chunk 4's column halves for cores 4-7. Every ordered (r,c)
    pair of the 8192x8192 matrix is covered exactly once across the
    fleet; per-core matmul volume is 4.0625 block-equivalents, the
    SPMD-uniform minimum at 128-col granularity.
  - PE runs fp8e4 DoubleRow matmuls (2 k-subtiles of 128 per pass: 256
    deep per 512-col pass, the TRN2 fp8 peak ~157 TF/s; measured 227ns
    per 128x256x512 instruction, narrow tails down to 80ns). The 1024
    contraction = 4 DoubleRow groups accumulated in PSUM. A dk-major,
    half-major warmup wave over block 0's first three m-tiles starts the
    PE on the first 128 KB col-half of the first dk slice.
  - Input DMA fans out over the three DMA-capable queues (sync/SP,
    gpsimd/Pool, scalar/ACT) in need order; the queues share ~360 GB/s,
    so the first dk slice is split across two queues and chunks stream
    in consumption order 0, 4, 1, 2, 3. Schedule order: warmup(m0-2),
    b0 m3-m4, b4, b1, b2, b3, then b0 m5/m6/m7 as a narrowing tail so
    the final exp is 128 cols and every late DMA overlaps matmuls.
  - Self-diag masked on DVE: a scalar_tensor_tensor adds -2000*I to the
    diag tile of G in PSUM before the exp (G_diag ~ +256 -> masked logit
    ~ -97 -> exp 0); gpsimd cannot access PSUM and the PE is the
    bottleneck, so DVE it is.
  - Positive pairs live on the diagonal supertiles of block 4, which only
    cores 0-3 compute; sim symmetry makes each such entry the positive
    for BOTH its row and its partner row, so the host credits core c's
    pos to rows of cores c and c+4.
  - Column-side reduction fully off-device.  Blocks 0/1/2/4: DVE
    accumulates sum_m E_m in bf16 and the [128, 1024] tile DMAs to HBM
    mid-run; block 0 credits only cols strictly right of each m-tile's
    diag tile (the diag tile is computed in both orders) and flushes in
    two pieces as its columns finalize.  Block 3 (the last full block)
    instead streams its raw E tiles out as they are produced: its DVE
    add chain + 256 KB acc DMA would otherwise end right before the
    epilogue drain.  The HOST collapses the partition axis everywhere
    (the ones-matmul partition reduction used in v2 cost ~3us of PE).
  - Tail choreography: the last tiles' credits flush on the sync queue,
    the final slots DMA descriptor rides the scalar queue directly
    behind m7's accumulator read (no cross-engine semaphore hop), and
    the last two e3 tiles split across sync+gpsimd so both queue drains
    clear while the 128-col final exp retires.
  - Host combines: per-row sumexp = own row-side + column-side chunks
    credited across cores; loss = mean(log(sumexp) - pos/(S^2*T)).
  - Typical HW exec: ~75.5-77us (PE window ~10.5..70.5us, stream ~59us
    at ~94% of the fp8 DoubleRow instruction-rate peak; baseline was
    86954ns).  Device clock state is bimodal: the same NEFF measures
    ~18% slower when the part downclocks; compare matmul slice rates
    before comparing exec times across runs.

This container's walrus build only accepts ONE semaphore wait per
instruction (and none on CTRL-encoded ones like Drain), while Tile freely
emits several. Three workarounds below: the TileContext epilogue drain's
waits are re-emitted on DVE memsets; a post-pass drops waits on a sem
that only the waiting instruction's own (in-order) engine increments and
whose target count was already reached earlier in that engine's stream;
and a second post-pass splits any remaining multi-wait instruction by
inserting single-wait event-semaphore "carrier" clones just before it on
the same engine stream (the same wait-only encoding Tile itself emits).
Both epilogue all_engine_barriers are load-bearing: without the final
one the NEFF's re-executions (NTFF profiling iterations) start with
dirty semaphore state and run ~15% slower.
"""

import copy

import numpy as np
import ml_dtypes


def _install_tile_drain_patch():
    import concourse.tile as tile
    from concourse import mybir
    from concourse.vector_clock import ScopedClock

    if getattr(tile.TileContext, "_drain_patch_installed", False):
        return

    def _drain_and_barrier(self, tick_clock, wait_clock):
        nc = self.nc
        drain_inst = nc.sync.drain()
        wait_clock.add_sem_waits(
            drain_inst.ins, ScopedClock({None: tick_clock.global_clock})
        )
        waits = list(drain_inst.ins.sync_info.on_wait)
        drain_inst.ins.sync_info.on_wait.clear()

        if waits:
            scr = nc.const_aps.tensor(0.0, (1, 1), mybir.dt.float32)
            for w in waits:
                ms = nc.vector.memset(scr, 0)
                if ms.ins.sync_info is None:
                    ms.ins.sync_info = mybir.SyncInfo(on_wait=[], on_update=[])
                ms.ins.sync_info.on_wait.append(w)

        nc.all_engine_barrier()
        assert self.sems is not None
        popped = nc._tile_sem_poison_stack.pop()
        assert popped is self._sem_poison
        nc.clear_and_free_semaphores(list(self.sems.allocated().values()))
        nc.all_engine_barrier()

    tile.TileContext._drain_and_barrier = _drain_and_barrier
    tile.TileContext._drain_patch_installed = True


_install_tile_drain_patch()

import concourse.bass as bass
import concourse.tile as tile
from concourse import mybir
from concourse.bass_utils import run_bass_kernel_spmd
from concourse.masks import make_identity

P = 128
D = 1024
R = 8192          # 2N rows
MY = 1024         # rows per core (= block size)
NB = 5            # column blocks computed per core (symmetry: 0..4)
KT = 8            # 128-deep k-subtiles in D
DKT = 4           # DoubleRow groups (256-deep each)
MT = 8            # m-tiles per core
CW = 1024         # column chunk width (= one block)
TEMP = 0.07
S = 16.0          # fp8 pre-scale; G = S^2 * sim
INVT2 = float(1.0 / (S * S * TEMP))
FP8 = mybir.dt.float8e4
BF16 = mybir.dt.bfloat16
F32 = mybir.dt.float32
ALU = mybir.AluOpType
ACTF = mybir.ActivationFunctionType
DR = mybir.MatmulPerfMode.DoubleRow

TRACE = False          # set True externally (test harness) for NTFF profiling
LAST_RESULTS = None    # BassKernelResults of the last run (for the harness)

_NC_CACHE = None


_COMPUTE_INSTS = {
    "InstMatmult", "InstLdweights", "InstActivation", "InstTensorTensor",
    "InstTensorScalarPtr", "InstTensorCopy", "InstMemset", "InstTensorReduce",
    "InstTensorScalarAffineSelect",
}


def _elide_self_waits(nc):
    """Drop semaphore waits that are trivially satisfied by same-engine
    program order: a wait on a sem that is only ever incremented by compute
    instructions of the waiting instruction's own engine, with a target
    value already reached by the updates of instructions earlier in that
    (in-order) engine stream."""
    updaters = {}       # sem id -> set of (engine, inst type)
    for f in nc.m.functions:
        for bb in f.blocks:
            for ins in bb.instructions:
                si = getattr(ins, "sync_info", None)
                if si is None:
                    continue
                for u in si.on_update:
                    if u.update_mode != "sem-inc":
                        updaters.setdefault(u.id, set()).add(("!", "!"))
                        continue
                    updaters.setdefault(u.id, set()).add(
                        (ins.engine, type(ins).__name__))
    self_sems = {}      # sem id -> engine (safe to elide for that engine)
    for sid, kinds in updaters.items():
        engines = {e for e, _ in kinds}
        types = {t for _, t in kinds}
        if len(engines) == 1 and types <= _COMPUTE_INSTS:
            self_sems[sid] = next(iter(engines))

    counts = {}         # (engine, sem id) -> completed increments so far
    n = 0
    for f in nc.m.functions:
        for bb in f.blocks:
            for ins in bb.instructions:
                si = getattr(ins, "sync_info", None)
                if si is None:
                    continue
                if si.on_wait:
                    keep = []
                    for w in si.on_wait:
                        eng = self_sems.get(w.id)
                        if (eng == ins.engine
                                and w.wait_mode == "sem-ge-imm"
                                and w.wait_value <= counts.get(
                                    (eng, w.id), 0)):
                            n += 1
                        else:
                            keep.append(w)
                    if len(keep) != len(si.on_wait):
                        del si.on_wait[:]
                        si.on_wait.extend(keep)
                for u in si.on_update:
                    if u.id in self_sems and self_sems[u.id] == ins.engine:
                        counts[(ins.engine, u.id)] = (
                            counts.get((ins.engine, u.id), 0) + u.update_value)
    return n


def _split_multi_waits(nc, templates):
    """Rewrite any instruction carrying >1 sem waits: keep the last wait,
    move each extra onto a fresh single-wait clone of the same-engine no-op
    template inserted immediately before it (engine streams are in-order)."""
    n = 0
    for f in nc.m.functions:
        for bb in f.blocks:
            newlist = []
            for ins in bb.instructions:
                si = getattr(ins, "sync_info", None)
                if si is not None and si.on_wait and len(si.on_wait) > 1:
                    extras = list(si.on_wait[:-1])
                    keep = list(si.on_wait[-1:])
                    tmpl = templates.get(ins.engine)
                    assert tmpl is not None, (
                        f"no wait-carrier template for engine {ins.engine} "
                        f"({type(ins).__name__} {ins.name})"
                    )
                    for w in extras:
                        c = copy.deepcopy(tmpl)
                        c.name = f"wcarrier-{n}"
                        n += 1
                        c.sync_info = mybir.SyncInfo(on_wait=[w], on_update=[])
                        newlist.append(c)
                    del si.on_wait[:]
                    si.on_wait.extend(keep)
                newlist.append(ins)
            bb.instructions[:] = newlist
    return n


def build():
    nc = bass.Bass()
    # [jc][p][ks][col]: element (p, ks, col) of chunk jc = q_rot[jc*CW+col,
    # ks*128+p]; flattened to rows jc*128+p, free ks*CW+col.
    zt = nc.dram_tensor("zt", [NB * P, KT, CW], FP8, kind="ExternalInput")
    slots_d = nc.dram_tensor("slots", [P, MT * NB], F32, kind="ExternalOutput")
    pos_d = nc.dram_tensor("pos", [P, MT], F32, kind="ExternalOutput")
    acc_d = nc.dram_tensor("accsum", [NB * P, CW], BF16, kind="ExternalOutput")
    e3_d = nc.dram_tensor("e3", [MT * P, CW], BF16, kind="ExternalOutput")

    templates = {}

    with tile.TileContext(nc) as tc:
        with (
            tc.tile_pool(name="singles", bufs=1) as singles,
            tc.tile_pool(name="epool", bufs=6) as epool,
            tc.tile_pool(name="psum_g", bufs=4, space="PSUM") as psum_g,
        ):
            zt_sb = [singles.tile([P, KT, CW], FP8, name=f"zt{j}")
                     for j in range(NB)]
            I128 = singles.tile([P, P], F32)
            slots = singles.tile([P, MT * NB], F32)
            pos = singles.tile([P, MT], F32)
            junk_pos = singles.tile([P, P], F32)
            acc = [singles.tile([P, CW], BF16, name=f"acc{b}")
                   for b in range(NB)]
            scr_a = singles.tile([1, 1], F32)

            c0 = nc.const_aps.tensor(0.0, (1, 1), F32)

            # --- input DMA fan-out over the three DMA-capable queues
            # (sync/SP, gpsimd/Pool, scalar/Activation).  The queues share
            # ~360 GB/s aggregate, so the very first slice (gating the
            # first matmul) is split into two 128 KB col-halves and the
            # other queues' first transfers are staggered behind cheap
            # engine work (identity / exp-table warm) so it rides alone.
            def dkslice(jc, dk, q):
                q.dma_start(
                    out=zt_sb[jc][:, 2 * dk:2 * dk + 2, :],
                    in_=zt[jc * P:(jc + 1) * P, 2 * dk:2 * dk + 2, :])

            nc.sync.dma_start(
                out=zt_sb[0][:, 0:2, 0:512], in_=zt[0:P, 0:2, 0:512])
            nc.scalar.dma_start(
                out=zt_sb[0][:, 0:2, 512:CW], in_=zt[0:P, 0:2, 512:CW])
            make_identity(nc, I128[:, :])      # gpsimd, before its descs
            # warm the ACT Exp table after scalar's first descriptor
            nc.scalar.activation(out=scr_a[:], in_=c0, func=ACTF.Exp)
            dkslice(0, 1, nc.gpsimd)
            dkslice(0, 2, nc.sync)
            dkslice(0, 3, nc.scalar)
            dkslice(4, 0, nc.sync)
            dkslice(4, 1, nc.gpsimd)
            dkslice(4, 2, nc.scalar)
            dkslice(4, 3, nc.gpsimd)
            nc.sync.dma_start(
                out=zt_sb[1][:, 0:4, :], in_=zt[P:2 * P, 0:4, :])
            nc.gpsimd.dma_start(
                out=zt_sb[1][:, 4:8, :], in_=zt[P:2 * P, 4:8, :])
            nc.scalar.dma_start(
                out=zt_sb[2][:, 0:4, :], in_=zt[2 * P:3 * P, 0:4, :])
            nc.scalar.dma_start(
                out=zt_sb[2][:, 4:8, :], in_=zt[2 * P:3 * P, 4:8, :])
            nc.sync.dma_start(
                out=zt_sb[3][:, 0:4, :], in_=zt[3 * P:4 * P, 0:4, :])
            nc.gpsimd.dma_start(
                out=zt_sb[3][:, 4:8, :], in_=zt[3 * P:4 * P, 4:8, :])

            # --- wait-carrier templates: bare event-semaphore instructions
            # (the same wait-only encoding Tile emits itself; far cheaper
            # than a compute no-op on the busy engines) ---
            tsem = nc.alloc_semaphore("wcarrier_dummy")
            for eng_name, eng_t in (("vector", mybir.EngineType.DVE),
                                    ("scalar", mybir.EngineType.Activation),
                                    ("gpsimd", mybir.EngineType.Pool),
                                    ("sync", mybir.EngineType.SP),
                                    ("tensor", mybir.EngineType.PE)):
                ins = getattr(nc, eng_name).wait_ge(tsem, 0).ins
                ins.sync_info.on_wait.clear()
                templates[eng_t] = ins

            def tri_geom(jc, m):
                # Returns (coff, [matmul chunk widths]).
                # block 0: a true upper triangle at 128-col granularity --
                # m-tile m computes only cols >= 128*m (own diag tile plus
                # everything right of it); chunk boundaries stay at col
                # 512 so the warmup can consume the first dk slice in two
                # halves.  Narrow matmuls are cheap (128-col ~80ns), far
                # cheaper than recomputing the lower triangle.
                # block 4: the half-block split (one 512-col supertile
                # per m-tile); blocks 1-3: full width.
                if jc == 0:
                    coff = m * P
                    if coff < 512:
                        return coff, ((512 - coff, 512) if coff else
                                      (512, 512))
                    return coff, (CW - coff,)
                if jc == 4:
                    return 512 * (m // 4), (512,)
                return 0, (512, 512)

            def emit_mms(jc, m, g):
                # g holds block coordinates: chunk [x, x+w) of the block
                # lands at g[:, x:x+w], so ragged chunk boundaries stay on
                # the PSUM bank line (col 512) -- a matmul output that
                # straddles a bank boundary is corrupted.
                coff, chunks = tri_geom(jc, m)
                for dk in range(DKT):
                    lhsT = zt_sb[0][:, 2 * dk:2 * dk + 2, m * P:(m + 1) * P]
                    x = coff
                    for w in chunks:
                        nc.tensor.matmul(
                            g[:, x:x + w],
                            lhsT,
                            zt_sb[jc][:, 2 * dk:2 * dk + 2, x:x + w],
                            start=(dk == 0), stop=(dk == DKT - 1),
                            perf_mode=DR,
                            skip_group_check=True)
                        x += w

            def emit_post(jc, m, g):
                coff, chunks = tri_geom(jc, m)
                W = sum(chunks)
                doff = m * P               # diag tile position (block coords)
                if jc == 0:
                    # self-diag -> big negative, on DVE (PE is the
                    # bottleneck; the extra exp wait is a cheap event;
                    # gpsimd cannot access PSUM)
                    nc.vector.scalar_tensor_tensor(
                        out=g[:, doff:doff + P], in0=I128[:], scalar=-2000.0,
                        in1=g[:, doff:doff + P], op0=ALU.mult, op1=ALU.add)
                if jc == 4:
                    # positive pair: rotated column = row + 4096 (reads g,
                    # doesn't block the exp).  Only meaningful on cores
                    # 0-3 (diag supertiles); garbage on 4-7, host ignores.
                    nc.vector.scalar_tensor_tensor(
                        out=junk_pos[:], in0=g[:, doff:doff + P], scalar=1.0,
                        in1=I128[:], op0=ALU.mult, op1=ALU.mult,
                        accum_out=pos[:, m:m + 1])
                e = epool.tile([P, CW], BF16, tag="e")
                nc.scalar.activation(
                    out=e[:, 0:W], in_=g[:, coff:coff + W], func=ACTF.Exp,
                    scale=INVT2,
                    accum_out=slots[:, jc * MT + m:jc * MT + m + 1])
                # column-side partial: acc_b += E_m (bf16, DVE).
                if jc == 0:
                    # credit everything strictly right of the own diag
                    # tile: those entries' transposes are in the never-
                    # computed lower triangle.  The diag tile itself is
                    # computed in both orders -> no credit.
                    lo = (m + 1) * P                      # global col start
                    if lo < CW:
                        src = e[:, lo - coff:W]
                        if m == 0:
                            nc.vector.tensor_copy(acc[0][:, lo:CW], src)
                        else:
                            nc.vector.tensor_tensor(
                                out=acc[0][:, lo:CW], in0=acc[0][:, lo:CW],
                                in1=src, op=ALU.add)
                elif jc == 3:
                    # last full block: its column sums would otherwise be
                    # a serialized DVE add chain ending right before the
                    # epilogue; stream the raw E tiles out instead and
                    # let the host reduce them (queues are idle by then).
                    if m < MT - 2:
                        q = nc.sync if m % 2 else nc.gpsimd
                        q.dma_start(
                            out=e3_d[m * P:(m + 1) * P, :], in_=e[:, :])
                    else:
                        # the last two tiles bound the epilogue drains:
                        # split each across both queues
                        nc.sync.dma_start(
                            out=e3_d[m * P:(m + 1) * P, 0:512],
                            in_=e[:, 0:512])
                        nc.gpsimd.dma_start(
                            out=e3_d[m * P:(m + 1) * P, 512:CW],
                            in_=e[:, 512:CW])
                elif jc == 4:
                    # half-block split: nothing is computed twice, so
                    # every entry credits its column.
                    dst = acc[4][:, coff:coff + 512]
                    if m % 4 == 0:
                        nc.vector.tensor_copy(dst, e[:, 0:512])
                    else:
                        nc.vector.tensor_tensor(
                            out=dst, in0=acc[4][:, coff:coff + 512],
                            in1=e[:, 0:512], op=ALU.add)
                else:
                    if m == 0:
                        nc.vector.tensor_copy(acc[jc][:], e[:])
                    else:
                        nc.vector.tensor_tensor(
                            out=acc[jc][:], in0=acc[jc][:],
                            in1=e[:], op=ALU.add)

            def emit_main(jc, m):
                g = psum_g.tile([P, CW], F32, tag="g")
                emit_mms(jc, m, g)
                emit_post(jc, m, g)

            # Warmup wave: the first three m-tiles of block 0 run dk-major
            # so matmuls start as soon as the first dk DMA slice lands
            # instead of waiting for the whole chunk.
            WU = 3
            gs = [psum_g.tile([P, CW], F32, tag="g", name=f"gwu{i}")
                  for i in range(WU)]
            for dk in range(DKT):
                # waves run half-major so the first three matmuls only
                # need the first 128 KB col-half of the first dk slice
                for half in (0, 1):
                    for mu in range(WU):
                        coff, chunks = tri_geom(0, mu)
                        x = coff if half == 0 else 512
                        w = chunks[half]
                        lhsT = zt_sb[0][:, 2 * dk:2 * dk + 2,
                                        mu * P:(mu + 1) * P]
                        nc.tensor.matmul(
                            gs[mu][:, x:x + w],
                            lhsT,
                            zt_sb[0][:, 2 * dk:2 * dk + 2, x:x + w],
                            start=(dk == 0), stop=(dk == DKT - 1),
                            perf_mode=DR,
                            skip_group_check=True)
            for mu in range(WU):
                emit_post(0, mu, gs[mu])

            # block 0's m3/m4 run right after the warmup (chunk-0-only
            # work while chunk 4 streams in); acc[0] cols 128:768 are
            # final after m4's credit.
            emit_main(0, 3)
            emit_main(0, 4)
            nc.gpsimd.dma_start(
                out=acc_d[0:P, P:768], in_=acc[0][:, P:768])

            for jc in (4, 1, 2, 3):
                for m in range(MT):
                    emit_main(jc, m)
                # stream this block's outputs as soon as they are complete
                nc.sync.dma_start(
                    out=slots_d[:, jc * MT:(jc + 1) * MT],
                    in_=slots[:, jc * MT:(jc + 1) * MT])
                if jc not in (0, 3):
                    nc.gpsimd.dma_start(
                        out=acc_d[jc * P:(jc + 1) * P, :], in_=acc[jc][:, :])
                if jc == 4:
                    nc.sync.dma_start(out=pos_d[:, :], in_=pos[:])

            # tail: block 0's m 5-7 are narrow (384/256/128) tiles, so
            # the final exps are short and block 3's acc DMAs + drain
            # overlap their matmuls.
            for m in range(5, MT):
                emit_main(0, m)
                if m == 6:
                    # acc[0] cols 768:1024 final after m6's credit (sync:
                    # a scalar-queue descriptor here would sit in the ACT
                    # stream and delay m7's exp)
                    nc.sync.dma_start(
                        out=acc_d[0:P, 768:CW], in_=acc[0][:, 768:CW])
            # final slots DMA on the scalar queue: its descriptor follows
            # m7's accumulator read on the SAME engine, skipping a cross-
            # engine semaphore hop on the critical tail chain
            nc.scalar.dma_start(
                out=slots_d[:, 0:MT], in_=slots[:, 0:MT])

    _elide_self_waits(nc)
    _split_multi_waits(nc, templates)
    return nc


def _prep_core_input(q8, c):
    """q8: [8192, 1024] fp8 (normalized*S). Returns the [640, 8192] fp8
    array for core c: rotated rows (own block first), first 5 blocks,
    k-subtile-major layout.  Cores 4-7 get chunk 4's column halves
    swapped (the anti-diagonal side of the block-4 supertile split)."""
    zr = np.roll(q8, -c * MY, axis=0)[:NB * MY]          # [5120, 1024]
    # chunk jc: [1024 cols][8 ks][128 p] -> [128 p][8 ks][1024 cols]
    a = zr.reshape(NB, CW, KT, P).transpose(0, 3, 2, 1)  # [5, 128, 8, 1024]
    a = np.ascontiguousarray(a)
    if c >= 4:
        a[4] = np.concatenate([a[4][..., 512:], a[4][..., :512]], axis=-1)
    return a.reshape(NB * P, KT, CW)


def kernel(z_i: np.ndarray, z_j: np.ndarray) -> np.ndarray:
    global _NC_CACHE, LAST_RESULTS
    z = np.concatenate([np.asarray(z_i, dtype=np.float32),
                        np.asarray(z_j, dtype=np.float32)], axis=0)
    norm = np.maximum(np.sqrt((z.astype(np.float64) ** 2).sum(axis=1,
                                                              keepdims=True)),
                      1e-8)
    q8 = ((z / norm) * S).astype(ml_dtypes.float8_e4m3)

    in_maps = [{"zt": _prep_core_input(q8, c)} for c in range(8)]

    if _NC_CACHE is None:
        _NC_CACHE = build()

    res = run_bass_kernel_spmd(
        _NC_CACHE, in_maps, core_ids=list(range(8)), trace=TRACE)
    LAST_RESULTS = res

    sumexp = np.zeros(R, np.float64)
    pos_g = np.zeros(R, np.float64)
    for c in range(8):
        slots = res.results[c]["slots"].astype(np.float64)   # [128, jc*8+m]
        rs = slots.reshape(P, NB, MT).sum(axis=1)            # [p, m]
        sumexp[c * MY:(c + 1) * MY] += rs.T.reshape(MY)      # row i = m*128+p
        if c < 4:
            # block-4 diag supertiles hold the positives for this core's
            # rows AND (by symmetry) its partner core's rows
            posv = res.results[c]["pos"].astype(np.float64)  # [p, m]
            v = posv.T.reshape(MY) * INVT2
            pos_g[c * MY:(c + 1) * MY] = v
            pos_g[(c + 4) * MY:(c + 5) * MY] = v
        accv = res.results[c]["accsum"].astype(np.float64)   # [640, 1024]
        e3 = res.results[c]["e3"].astype(np.float64)         # [1024, 1024]
        for b in range(NB):
            if b == 3:
                colb = e3.sum(axis=0)    # raw E tiles, host-reduced
            else:
                colb = accv[b * P:(b + 1) * P, :].sum(axis=0)
            if b == 0:
                colb[:P] = 0.0           # never written on device
            if b == 4 and c >= 4:
                # undo the host-side column-half swap of chunk 4
                colb = np.concatenate([colb[512:], colb[:512]])
            gb = (c + b) % 8
            sumexp[gb * MY:(gb + 1) * MY] += colb
    loss = np.mean(np.log(sumexp) - pos_g)
    return np.float32(loss)


# revision 44
# speedup vs baseline: 1.1841x; 1.1841x over previous
"""NT-Xent contrastive loss (SimCLR) on 8 Trainium2 NeuronCores.

Strategy (v6: host-norm + fp8 DoubleRow + 128-granular symmetry):
  - Host: z = concat(z_i, z_j) [8192, 1024], L2-normalize rows in f32,
    scale by S=16, quantize to fp8 e4m3. The cosine-sim matrix is then
    just G = q @ q.T (scaled by S^2), no on-device normalization.
  - Symmetry: sim is symmetric. In rotated coords (each core's 1024 rows
    at block 0), core c computes column blocks 1-3 in full; block 0 (its
    own diagonal block) as a TRUE upper triangle at 128-col granularity
    (m-tile m computes cols >= 128*m, in ragged 512/384/256/128-col
    chunks whose boundaries stay on the PSUM bank line at col 512 -- a
    matmul output straddling a bank boundary is silently corrupted); and
    block 4 as HALF a block: the (c, c+4) / (c+4, c) pair is split at
    512-supertile granularity - cores 0-3 compute the two diagonal
    supertiles, cores 4-7 the two anti-diagonal ones. SPMD runs ONE
    program on all cores, so that split is realized in DATA: the host
    swaps chunk 4's column halves for cores 4-7. Every ordered (r,c)
    pair of the 8192x8192 matrix is covered exactly once across the
    fleet; per-core matmul volume is 4.0625 block-equivalents, the
    SPMD-uniform minimum at 128-col granularity.
  - PE runs fp8e4 DoubleRow matmuls (2 k-subtiles of 128 per pass: 256
    deep per 512-col pass, the TRN2 fp8 peak ~157 TF/s; measured 227ns
    per 128x256x512 instruction, narrow tails down to 80ns). The 1024
    contraction = 4 DoubleRow groups accumulated in PSUM. A dk-major,
    half-major warmup wave over block 0's first three m-tiles starts the
    PE on the first 128 KB col-half of the first dk slice.
  - Input DMA fans out over the three DMA-capable queues (sync/SP,
    gpsimd/Pool, scalar/ACT) in need order; the queues share ~360 GB/s,
    so the first dk slice is split across two queues and chunks stream
    in consumption order 0, 4, 1, 2, 3. Schedule order: warmup(m0-2),
    b0 m3-m4, b4, b1, b2, b3, then b0 m5/m6/m7 as a narrowing tail so
    the final exp is 128 cols and every late DMA overlaps matmuls.
  - Self-diag masked on DVE: a scalar_tensor_tensor adds -2000*I to the
    diag tile of G in PSUM before the exp (G_diag ~ +256 -> masked logit
    ~ -97 -> exp 0); gpsimd cannot access PSUM and the PE is the
    bottleneck, so DVE it is.
  - Positive pairs live on the diagonal supertiles of block 4, which only
    cores 0-3 compute; sim symmetry makes each such entry the positive
    for BOTH its row and its partner row, so the host credits core c's
    pos to rows of cores c and c+4.
  - Column-side reduction fully off-device.  Blocks 0/1/2/4: DVE
    accumulates sum_m E_m in bf16 and the [128, 1024] tile DMAs to HBM
    mid-run; block 0 credits only cols strictly right of each m-tile's
    diag tile (the diag tile is computed in both orders) and flushes in
    two pieces as its columns finalize.  Block 3 (the last full block)
    instead streams its raw E tiles out as they are produced: its DVE
    add chain + 256 KB acc DMA would otherwise end right before the
    epilogue drain.  The HOST collapses the partition axis everywhere
    (the ones-matmul partition reduction used in v2 cost ~3us of PE).
  - Tail choreography: the last tiles' credits flush on the sync queue,
    the final slots DMA descriptor rides the scalar queue directly
    behind m7's accumulator read (no cross-engine semaphore hop), and
    the last two e3 tiles split across sync+gpsimd so both queue drains
    clear while the 128-col final exp retires.
  - Host combines: per-row sumexp = own row-side + column-side chunks
    credited across cores; loss = mean(log(sumexp) - pos/(S^2*T)).
  - Typical HW exec: ~75.5-77us (PE window ~10.5..70.5us, stream ~59us
    at ~94% of the fp8 DoubleRow instruction-rate peak; baseline was
    86954ns).  Device clock state is bimodal: the same NEFF measures
    ~18% slower when the part downclocks; compare matmul slice rates
    before comparing exec times across runs.

This container's walrus build only accepts ONE semaphore wait per
instruction (and none on CTRL-encoded ones like Drain), while Tile freely
emits several. Three workarounds below: the TileContext epilogue drain's
waits are re-emitted on DVE memsets; a post-pass drops waits on a sem
that only the waiting instruction's own (in-order) engine increments and
whose target count was already reached earlier in that engine's stream;
and a second post-pass splits any remaining multi-wait instruction by
inserting single-wait event-semaphore "carrier" clones just before it on
the same engine stream (the same wait-only encoding Tile itself emits).
Both epilogue all_engine_barriers are load-bearing: without the final
one the NEFF's re-executions (NTFF profiling iterations) start with
dirty semaphore state and run ~15% slower.
"""

import copy

import numpy as np
import ml_dtypes


def _install_tile_drain_patch():
    import concourse.tile as tile
    from concourse import mybir
    from concourse.vector_clock import ScopedClock

    if getattr(tile.TileContext, "_drain_patch_installed", False):
        return

    def _drain_and_barrier(self, tick_clock, wait_clock):
        nc = self.nc
        drain_inst = nc.sync.drain()
        wait_clock.add_sem_waits(
            drain_inst.ins, ScopedClock({None: tick_clock.global_clock})
        )
        waits = list(drain_inst.ins.sync_info.on_wait)
        drain_inst.ins.sync_info.on_wait.clear()

        if waits:
            scr = nc.const_aps.tensor(0.0, (1, 1), mybir.dt.float32)
            for w in waits:
                ms = nc.vector.memset(scr, 0)
                if ms.ins.sync_info is None:
                    ms.ins.sync_info = mybir.SyncInfo(on_wait=[], on_update=[])
                ms.ins.sync_info.on_wait.append(w)

        nc.all_engine_barrier()
        assert self.sems is not None
        popped = nc._tile_sem_poison_stack.pop()
        assert popped is self._sem_poison
        nc.clear_and_free_semaphores(list(self.sems.allocated().values()))
        nc.all_engine_barrier()

    tile.TileContext._drain_and_barrier = _drain_and_barrier
    tile.TileContext._drain_patch_installed = True


_install_tile_drain_patch()

import concourse.bass as bass
import concourse.tile as tile
from concourse import mybir
from concourse.bass_utils import run_bass_kernel_spmd
from concourse.masks import make_identity

P = 128
D = 1024
R = 8192          # 2N rows
MY = 1024         # rows per core (= block size)
NB = 5            # column blocks computed per core (symmetry: 0..4)
KT = 8            # 128-deep k-subtiles in D
DKT = 4           # DoubleRow groups (256-deep each)
MT = 8            # m-tiles per core
CW = 1024         # column chunk width (= one block)
TEMP = 0.07
S = 16.0          # fp8 pre-scale; G = S^2 * sim
INVT2 = float(1.0 / (S * S * TEMP))
FP8 = mybir.dt.float8e4
BF16 = mybir.dt.bfloat16
F32 = mybir.dt.float32
ALU = mybir.AluOpType
ACTF = mybir.ActivationFunctionType
DR = mybir.MatmulPerfMode.DoubleRow

TRACE = False          # set True externally (test harness) for NTFF profiling
LAST_RESULTS = None    # BassKernelResults of the last run (for the harness)

_NC_CACHE = None


_COMPUTE_INSTS = {
    "InstMatmult", "InstLdweights", "InstActivation", "InstTensorTensor",
    "InstTensorScalarPtr", "InstTensorCopy", "InstMemset", "InstTensorReduce",
    "InstTensorScalarAffineSelect",
}


def _elide_self_waits(nc):
    """Drop semaphore waits that are trivially satisfied by same-engine
    program order: a wait on a sem that is only ever incremented by compute
    instructions of the waiting instruction's own engine, with a target
    value already reached by the updates of instructions earlier in that
    (in-order) engine stream."""
    updaters = {}       # sem id -> set of (engine, inst type)
    for f in nc.m.functions:
        for bb in f.blocks:
            for ins in bb.instructions:
                si = getattr(ins, "sync_info", None)
                if si is None:
                    continue
                for u in si.on_update:
                    if u.update_mode != "sem-inc":
                        updaters.setdefault(u.id, set()).add(("!", "!"))
                        continue
                    updaters.setdefault(u.id, set()).add(
                        (ins.engine, type(ins).__name__))
    self_sems = {}      # sem id -> engine (safe to elide for that engine)
    for sid, kinds in updaters.items():
        engines = {e for e, _ in kinds}
        types = {t for _, t in kinds}
        if len(engines) == 1 and types <= _COMPUTE_INSTS:
            self_sems[sid] = next(iter(engines))

    counts = {}         # (engine, sem id) -> completed increments so far
    n = 0
    for f in nc.m.functions:
        for bb in f.blocks:
            for ins in bb.instructions:
                si = getattr(ins, "sync_info", None)
                if si is None:
                    continue
                if si.on_wait:
                    keep = []
                    for w in si.on_wait:
                        eng = self_sems.get(w.id)
                        if (eng == ins.engine
                                and w.wait_mode == "sem-ge-imm"
                                and w.wait_value <= counts.get(
                                    (eng, w.id), 0)):
                            n += 1
                        else:
                            keep.append(w)
                    if len(keep) != len(si.on_wait):
                        del si.on_wait[:]
                        si.on_wait.extend(keep)
                for u in si.on_update:
                    if u.id in self_sems and self_sems[u.id] == ins.engine:
                        counts[(ins.engine, u.id)] = (
                            counts.get((ins.engine, u.id), 0) + u.update_value)
    return n


def _split_multi_waits(nc, templates):
    """Rewrite any instruction carrying >1 sem waits: keep the last wait,
    move each extra onto a fresh single-wait clone of the same-engine no-op
    template inserted immediately before it (engine streams are in-order)."""
    n = 0
    for f in nc.m.functions:
        for bb in f.blocks:
            newlist = []
            for ins in bb.instructions:
                si = getattr(ins, "sync_info", None)
                if si is not None and si.on_wait and len(si.on_wait) > 1:
                    extras = list(si.on_wait[:-1])
                    keep = list(si.on_wait[-1:])
                    tmpl = templates.get(ins.engine)
                    assert tmpl is not None, (
                        f"no wait-carrier template for engine {ins.engine} "
                        f"({type(ins).__name__} {ins.name})"
                    )
                    for w in extras:
                        c = copy.deepcopy(tmpl)
                        c.name = f"wcarrier-{n}"
                        n += 1
                        c.sync_info = mybir.SyncInfo(on_wait=[w], on_update=[])
                        newlist.append(c)
                    del si.on_wait[:]
                    si.on_wait.extend(keep)
                newlist.append(ins)
            bb.instructions[:] = newlist
    return n


def build():
    nc = bass.Bass()
    # [jc][p][ks][col]: element (p, ks, col) of chunk jc = q_rot[jc*CW+col,
    # ks*128+p]; flattened to rows jc*128+p, free ks*CW+col.
    zt = nc.dram_tensor("zt", [NB * P, KT, CW], FP8, kind="ExternalInput")
    slots_d = nc.dram_tensor("slots", [P, MT * NB], F32, kind="ExternalOutput")
    pos_d = nc.dram_tensor("pos", [P, MT], F32, kind="ExternalOutput")
    acc_d = nc.dram_tensor("accsum", [NB * P, CW], BF16, kind="ExternalOutput")
    e3_d = nc.dram_tensor("e3", [MT * P, CW], BF16, kind="ExternalOutput")

    templates = {}

    with tile.TileContext(nc) as tc:
        with (
            tc.tile_pool(name="singles", bufs=1) as singles,
            tc.tile_pool(name="epool", bufs=6) as epool,
            tc.tile_pool(name="psum_g", bufs=4, space="PSUM") as psum_g,
        ):
            zt_sb = [singles.tile([P, KT, CW], FP8, name=f"zt{j}")
                     for j in range(NB)]
            I128 = singles.tile([P, P], F32)
            slots = singles.tile([P, MT * NB], F32)
            pos = singles.tile([P, MT], F32)
            junk_pos = singles.tile([P, P], F32)
            acc = [singles.tile([P, CW], BF16, name=f"acc{b}")
                   for b in range(NB)]
            scr_a = singles.tile([1, 1], F32)

            c0 = nc.const_aps.tensor(0.0, (1, 1), F32)

            # --- input DMA fan-out over the three DMA-capable queues
            # (sync/SP, gpsimd/Pool, scalar/Activation).  The queues share
            # ~360 GB/s aggregate, so the very first slice (gating the
            # first matmul) is split into two 128 KB col-halves and the
            # other queues' first transfers are staggered behind cheap
            # engine work (identity / exp-table warm) so it rides alone.
            def dkslice(jc, dk, q):
                q.dma_start(
                    out=zt_sb[jc][:, 2 * dk:2 * dk + 2, :],
                    in_=zt[jc * P:(jc + 1) * P, 2 * dk:2 * dk + 2, :])

            nc.sync.dma_start(
                out=zt_sb[0][:, 0:2, 0:512], in_=zt[0:P, 0:2, 0:512])
            nc.scalar.dma_start(
                out=zt_sb[0][:, 0:2, 512:CW], in_=zt[0:P, 0:2, 512:CW])
            make_identity(nc, I128[:, :])      # gpsimd, before its descs
            # warm the ACT Exp table after scalar's first descriptor
            nc.scalar.activation(out=scr_a[:], in_=c0, func=ACTF.Exp)
            dkslice(0, 1, nc.gpsimd)
            dkslice(0, 2, nc.sync)
            dkslice(0, 3, nc.scalar)
            dkslice(4, 0, nc.sync)
            dkslice(4, 1, nc.gpsimd)
            dkslice(4, 2, nc.scalar)
            dkslice(4, 3, nc.gpsimd)
            nc.sync.dma_start(
                out=zt_sb[1][:, 0:4, :], in_=zt[P:2 * P, 0:4, :])
            nc.gpsimd.dma_start(
                out=zt_sb[1][:, 4:8, :], in_=zt[P:2 * P, 4:8, :])
            nc.scalar.dma_start(
                out=zt_sb[2][:, 0:4, :], in_=zt[2 * P:3 * P, 0:4, :])
            nc.scalar.dma_start(
                out=zt_sb[2][:, 4:8, :], in_=zt[2 * P:3 * P, 4:8, :])
            nc.sync.dma_start(
                out=zt_sb[3][:, 0:4, :], in_=zt[3 * P:4 * P, 0:4, :])
            nc.gpsimd.dma_start(
                out=zt_sb[3][:, 4:8, :], in_=zt[3 * P:4 * P, 4:8, :])

            # --- wait-carrier templates: bare event-semaphore instructions
            # (the same wait-only encoding Tile emits itself; far cheaper
            # than a compute no-op on the busy engines) ---
            tsem = nc.alloc_semaphore("wcarrier_dummy")
            for eng_name, eng_t in (("vector", mybir.EngineType.DVE),
                                    ("scalar", mybir.EngineType.Activation),
                                    ("gpsimd", mybir.EngineType.Pool),
                                    ("sync", mybir.EngineType.SP),
                                    ("tensor", mybir.EngineType.PE)):
                ins = getattr(nc, eng_name).wait_ge(tsem, 0).ins
                ins.sync_info.on_wait.clear()
                templates[eng_t] = ins

            def tri_geom(jc, m):
                # Returns (coff, [matmul chunk widths]).
                # block 0: a true upper triangle at 128-col granularity --
                # m-tile m computes only cols >= 128*m (own diag tile plus
                # everything right of it); chunk boundaries stay at col
                # 512 so the warmup can consume the first dk slice in two
                # halves.  Narrow matmuls are cheap (128-col ~80ns), far
                # cheaper than recomputing the lower triangle.
                # block 4: the half-block split (one 512-col supertile
                # per m-tile); blocks 1-3: full width.
                if jc == 0:
                    coff = m * P
                    if coff < 512:
                        return coff, ((512 - coff, 512) if coff else
                                      (512, 512))
                    return coff, (CW - coff,)
                if jc == 4:
                    return 512 * (m // 4), (512,)
                return 0, (512, 512)

            def emit_mms(jc, m, g):
                # g holds block coordinates: chunk [x, x+w) of the block
                # lands at g[:, x:x+w], so ragged chunk boundaries stay on
                # the PSUM bank line (col 512) -- a matmul output that
                # straddles a bank boundary is corrupted.
                coff, chunks = tri_geom(jc, m)
                for dk in range(DKT):
                    lhsT = zt_sb[0][:, 2 * dk:2 * dk + 2, m * P:(m + 1) * P]
                    x = coff
                    for w in chunks:
                        nc.tensor.matmul(
                            g[:, x:x + w],
                            lhsT,
                            zt_sb[jc][:, 2 * dk:2 * dk + 2, x:x + w],
                            start=(dk == 0), stop=(dk == DKT - 1),
                            perf_mode=DR,
                            skip_group_check=True)
                        x += w

            def emit_post(jc, m, g):
                coff, chunks = tri_geom(jc, m)
                W = sum(chunks)
                doff = m * P               # diag tile position (block coords)
                if jc == 0:
                    # self-diag -> big negative, on DVE (PE is the
                    # bottleneck; the extra exp wait is a cheap event;
                    # gpsimd cannot access PSUM)
                    nc.vector.scalar_tensor_tensor(
                        out=g[:, doff:doff + P], in0=I128[:], scalar=-2000.0,
                        in1=g[:, doff:doff + P], op0=ALU.mult, op1=ALU.add)
                if jc == 4:
                    # positive pair: rotated column = row + 4096 (reads g,
                    # doesn't block the exp).  Only meaningful on cores
                    # 0-3 (diag supertiles); garbage on 4-7, host ignores.
                    nc.vector.scalar_tensor_tensor(
                        out=junk_pos[:], in0=g[:, doff:doff + P], scalar=1.0,
                        in1=I128[:], op0=ALU.mult, op1=ALU.mult,
                        accum_out=pos[:, m:m + 1])
                e = epool.tile([P, CW], BF16, tag="e")
                nc.scalar.activation(
                    out=e[:, 0:W], in_=g[:, coff:coff + W], func=ACTF.Exp,
                    scale=INVT2,
                    accum_out=slots[:, jc * MT + m:jc * MT + m + 1])
                # column-side partial: acc_b += E_m (bf16, DVE).
                if jc == 0:
                    # credit everything strictly right of the own diag
                    # tile: those entries' transposes are in the never-
                    # computed lower triangle.  The diag tile itself is
                    # computed in both orders -> no credit.
                    lo = (m + 1) * P                      # global col start
                    if lo < CW:
                        src = e[:, lo - coff:W]
                        if m == 0:
                            nc.vector.tensor_copy(acc[0][:, lo:CW], src)
                        else:
                            nc.vector.tensor_tensor(
                                out=acc[0][:, lo:CW], in0=acc[0][:, lo:CW],
                                in1=src, op=ALU.add)
                elif jc == 3:
                    # last full block: its column sums would otherwise be
                    # a serialized DVE add chain ending right before the
                    # epilogue; stream the raw E tiles out instead and
                    # let the host reduce them (queues are idle by then).
                    if m < MT - 2:
                        q = nc.sync if m % 2 else nc.gpsimd
                        q.dma_start(
                            out=e3_d[m * P:(m + 1) * P, :], in_=e[:, :])
                    else:
                        # the last two tiles bound the epilogue drains:
                        # split each across both queues
                        nc.sync.dma_start(
                            out=e3_d[m * P:(m + 1) * P, 0:512],
                            in_=e[:, 0:512])
                        nc.gpsimd.dma_start(
                            out=e3_d[m * P:(m + 1) * P, 512:CW],
                            in_=e[:, 512:CW])
                elif jc == 4:
                    # half-block split: nothing is computed twice, so
                    # every entry credits its column.
                    dst = acc[4][:, coff:coff + 512]
                    if m % 4 == 0:
                        nc.vector.tensor_copy(dst, e[:, 0:512])
                    else:
                        nc.vector.tensor_tensor(
                            out=dst, in0=acc[4][:, coff:coff + 512],
                            in1=e[:, 0:512], op=ALU.add)
                else:
                    if m == 0:
                        nc.vector.tensor_copy(acc[jc][:], e[:])
                    else:
                        nc.vector.tensor_tensor(
                            out=acc[jc][:], in0=acc[jc][:],
                            in1=e[:], op=ALU.add)

            def emit_main(jc, m):
                g = psum_g.tile([P, CW], F32, tag="g")
                emit_mms(jc, m, g)
                emit_post(jc, m, g)

            # Warmup wave: the first three m-tiles of block 0 run dk-major
            # so matmuls start as soon as the first dk DMA slice lands
            # instead of waiting for the whole chunk.
            WU = 3
            gs = [psum_g.tile([P, CW], F32, tag="g", name=f"gwu{i}")
                  for i in range(WU)]
            for dk in range(DKT):
                # waves run half-major so the first three matmuls only
                # need the first 128 KB col-half of the first dk slice
                for half in (0, 1):
                    for mu in range(WU):
                        coff, chunks = tri_geom(0, mu)
                        x = coff if half == 0 else 512
                        w = chunks[half]
                        lhsT = zt_sb[0][:, 2 * dk:2 * dk + 2,
                                        mu * P:(mu + 1) * P]
                        nc.tensor.matmul(
                            gs[mu][:, x:x + w],
                            lhsT,
                            zt_sb[0][:, 2 * dk:2 * dk + 2, x:x + w],
                            start=(dk == 0), stop=(dk == DKT - 1),
                            perf_mode=DR,
                            skip_group_check=True)
            for mu in range(WU):
                emit_post(0, mu, gs[mu])

            # block 0's m3/m4 run right after the warmup (chunk-0-only
            # work while chunk 4 streams in); acc[0] cols 128:768 are
            # final after m4's credit.
            emit_main(0, 3)
            emit_main(0, 4)
            nc.gpsimd.dma_start(
                out=acc_d[0:P, P:768], in_=acc[0][:, P:768])

            for jc in (4, 1, 2, 3):
                for m in range(MT):
                    emit_main(jc, m)
                # stream this block's outputs as soon as they are complete
                nc.sync.dma_start(
                    out=slots_d[:, jc * MT:(jc + 1) * MT],
                    in_=slots[:, jc * MT:(jc + 1) * MT])
                if jc not in (0, 3):
                    nc.gpsimd.dma_start(
                        out=acc_d[jc * P:(jc + 1) * P, :], in_=acc[jc][:, :])
                if jc == 4:
                    nc.sync.dma_start(out=pos_d[:, :], in_=pos[:])

            # tail: block 0's m 5-7 are narrow (384/256/128) tiles, so
            # the final exps are short and block 3's acc DMAs + drain
            # overlap their matmuls.
            for m in range(5, MT):
                emit_main(0, m)
                if m == 6:
                    # acc[0] cols 768:1024 final after m6's credit (sync:
                    # a scalar-queue descriptor here would sit in the ACT
                    # stream and delay m7's exp)
                    nc.sync.dma_start(
                        out=acc_d[0:P, 768:CW], in_=acc[0][:, 768:CW])
            # final slots DMA on the scalar queue: its descriptor follows
            # m7's accumulator read on the SAME engine, skipping a cross-
            # engine semaphore hop on the critical tail chain
            nc.scalar.dma_start(
                out=slots_d[:, 0:MT], in_=slots[:, 0:MT])

    _elide_self_waits(nc)
    _split_multi_waits(nc, templates)
    return nc


def _prep_core_input(q8, c):
    """q8: [8192, 1024] fp8 (normalized*S). Returns the [640, 8192] fp8
    array for core c: rotated rows (own block first), first 5 blocks,
    k-subtile-major layout.  Cores 4-7 get chunk 4's column halves
    swapped (the anti-diagonal side of the block-4 supertile split)."""
    zr = np.roll(q8, -c * MY, axis=0)[:NB * MY]          # [5120, 1024]
    # chunk jc: [1024 cols][8 ks][128 p] -> [128 p][8 ks][1024 cols]
    a = zr.reshape(NB, CW, KT, P).transpose(0, 3, 2, 1)  # [5, 128, 8, 1024]
    a = np.ascontiguousarray(a)
    if c >= 4:
        a[4] = np.concatenate([a[4][..., 512:], a[4][..., :512]], axis=-1)
    return a.reshape(NB * P, KT, CW)


def kernel(z_i: np.ndarray, z_j: np.ndarray) -> np.ndarray:
    global _NC_CACHE, LAST_RESULTS
    z = np.concatenate([np.asarray(z_i, dtype=np.float32),
                        np.asarray(z_j, dtype=np.float32)], axis=0)
    norm = np.maximum(np.sqrt((z.astype(np.float64) ** 2).sum(axis=1,
                                                              keepdims=True)),
                      1e-8)
    q8 = ((z / norm) * S).astype(ml_dtypes.float8_e4m3)

    in_maps = [{"zt": _prep_core_input(q8, c)} for c in range(8)]

    if _NC_CACHE is None:
        _NC_CACHE = build()

    res = run_bass_kernel_spmd(
        _NC_CACHE, in_maps, core_ids=list(range(8)), trace=TRACE)
    LAST_RESULTS = res

    sumexp = np.zeros(R, np.float64)
    pos_g = np.zeros(R, np.float64)
    for c in range(8):
        slots = res.results[c]["slots"].astype(np.float64)   # [128, jc*8+m]
        rs = slots.reshape(P, NB, MT).sum(axis=1)            # [p, m]
        sumexp[c * MY:(c + 1) * MY] += rs.T.reshape(MY)      # row i = m*128+p
        if c < 4:
            # block-4 diag supertiles hold the positives for this core's
            # rows AND (by symmetry) its partner core's rows
            posv = res.results[c]["pos"].astype(np.float64)  # [p, m]
            v = posv.T.reshape(MY) * INVT2
            pos_g[c * MY:(c + 1) * MY] = v
            pos_g[(c + 4) * MY:(c + 5) * MY] = v
        accv = res.results[c]["accsum"].astype(np.float64)   # [640, 1024]
        e3 = res.results[c]["e3"].astype(np.float64)         # [1024, 1024]
        for b in range(NB):
            if b == 3:
                colb = e3.sum(axis=0)    # raw E tiles, host-reduced
            else:
                colb = accv[b * P:(b + 1) * P, :].sum(axis=0)
            if b == 0:
                colb[:P] = 0.0           # never written on device
            if b == 4 and c >= 4:
                # undo the host-side column-half swap of chunk 4
                colb = np.concatenate([colb[512:], colb[:512]])
            gb = (c + b) % 8
            sumexp[gb * MY:(gb + 1) * MY] += colb
    loss = np.mean(np.log(sumexp) - pos_g)
    return np.float32(loss)
